# revision 24
# baseline (speedup 1.0000x reference)
"""Multi-head self-attention (diag-zero mask) TRN2 kernel, 8-core head-parallel.

Sharding: 16 heads / 8 cores = 2 heads per core; every core sees the full
sequence (both batches), computes Q/K/V projections for its 2 heads,
attention, and its partial out-projection (Wo rows for its head block).
Host sums the 8 partial outputs (the out_proj all-reduce) and adds biases.

Math notes:
  - 1/sqrt(Dh) folded into Wq/bq on host.
  - scores are computed transposed (keys on partitions, queries on free dim)
    so exp() needs no on-chip reduction.
  - softmax denominator Z comes from a ones-vector matmul over the exp'd
    attention tiles, accumulated in PSUM like the A@V product.
  - diag-zero mask: multiply the score diagonal block by (1-eye) before exp
    (masked score 0 -> exp(0) = 1, matching the reference softmax).
  - bv and bo contributions are rank-1/constant terms folded in on host:
    out += bv @ Wo.T + bo.

Schedule (v4): ACT (exp, ~135us/core) is the target bottleneck; every other
engine's work is pipelined under it.
  - score matmuls are K=64 pairs on disjoint PE row groups (concurrent);
  - A@V matmuls are 64-dim-out pairs on disjoint PE col groups (concurrent);
  - Z matmuls are 1-row-out pairs on disjoint col groups;
  - projections and out-projections are spliced as filler into the
    attention stream's PE slack, so only ~1 m-tile of projection runs
    un-overlapped at kernel start.
Out-projection partials are emitted in bf16 (summed on host in f64).
"""

from collections import deque
from contextlib import ExitStack

import numpy as np
import ml_dtypes

import concourse.bass as bass
import concourse.tile as tile
from concourse import bacc, mybir
from concourse.bass_utils import run_bass_kernel_spmd

BF16 = mybir.dt.bfloat16
F32 = mybir.dt.float32

B = 2
D = 1024
H = 16
DH = 64
NCORES = 8
HLOC = H // NCORES          # 2 heads per core
DLOC = HLOC * DH            # 128 head-dims per core
KC = D // 128               # 8 contraction chunks for projections
MMW = 512                   # matmul moving width (one PSUM bank of f32)


def emit_kernel(tc, M, xT, wqT, wkT, wvT, woT, bq, bk, mask, out, dbg=None):
    """Emit the per-core program. M = per-batch sequence length."""
    nc = tc.nc
    S = B * M               # flattened sequence rows
    NKT = M // 128          # key tiles per batch
    NQT = M // MMW          # 512-wide q tiles per batch
    NMT = M // 128          # 128-row out-proj tiles per batch
    NJ = D // MMW

    with ExitStack() as ctx:
        consts = ctx.enter_context(tc.tile_pool(name="consts", bufs=1))
        QT = consts.tile([128, S], BF16)    # [2 heads x 64 dims, S]
        KT = consts.tile([128, S], BF16)
        Wo_sb = consts.tile([128, D], BF16)
        bq_sb = consts.tile([128, 1], F32)
        bk_sb = consts.tile([128, 1], F32)
        mask_sb = consts.tile([128, 128], F32)

        nc.sync.dma_start(Wo_sb, woT.ap())
        nc.sync.dma_start(bq_sb, bq.ap())
        nc.sync.dma_start(bk_sb, bk.ap())
        nc.sync.dma_start(mask_sb, mask.ap())

        # V natural layout + ones column: staging tiles written by DMA
        # transpose ([128,128] aligned), then DVE-copied into the 65-col
        # V1 layout (ones col appended for the free softmax denominator).
        V1 = consts.tile([128, B, NKT, HLOC, 65], BF16)
        nc.vector.memset(V1[:, :, :, :, 64:65], 1.0)
        vp = ctx.enter_context(tc.tile_pool(name="v_pool", bufs=4))

        # attention pools (live for the whole kernel)
        stp = ctx.enter_context(tc.tile_pool(name="st_psum", bufs=2, space="PSUM"))
        atp = ctx.enter_context(tc.tile_pool(name="at_pool", bufs=NKT + 2))
        ctp = ctx.enter_context(tc.tile_pool(name="ct_psum", bufs=2, space="PSUM"))
        rzp = ctx.enter_context(tc.tile_pool(name="rz_pool", bufs=4))
        rzbp = ctx.enter_context(tc.tile_pool(name="rzb_pool", bufs=4))
        cp = ctx.enter_context(tc.tile_pool(name="c_pool", bufs=B))
        C_tiles = {}

        # projection pools
        proj_sbuf = ctx.enter_context(tc.tile_pool(name="proj_sbuf", bufs=1))
        Wq_sb = proj_sbuf.tile([128, KC, DLOC], BF16)
        Wk_sb = proj_sbuf.tile([128, KC, DLOC], BF16)
        Wv_sb = proj_sbuf.tile([128, KC, DLOC], BF16)
        nc.sync.dma_start(Wk_sb, wkT.ap().rearrange("(c p) d -> p c d", p=128))
        nc.sync.dma_start(Wv_sb, wvT.ap().rearrange("(c p) d -> p c d", p=128))
        nc.sync.dma_start(Wq_sb, wqT.ap().rearrange("(c p) d -> p c d", p=128))
        X = proj_sbuf.tile([128, KC, S], BF16)
        xT_r = xT.ap().rearrange("(c p) m -> p c m", p=128)
        for mt in range(S // MMW):
            nc.sync.dma_start(
                X[:, :, mt * MMW:(mt + 1) * MMW],
                xT_r[:, :, mt * MMW:(mt + 1) * MMW],
            )
        VT = proj_sbuf.tile([128, S], BF16)

        pp = ctx.enter_context(tc.tile_pool(name="fill_psum", bufs=2, space="PSUM"))
        opp = pp
        osp = ctx.enter_context(tc.tile_pool(name="out_sbuf", bufs=4))

        def proj_tile(b, mt_loc, which):
            """One projection (K, V or Q) for one 512-wide m-tile of batch b.
            For V, also the DMA-transposes into the V tiles."""
            mt = b * NQT + mt_loc
            W_sb, dst, bias_sb = {
                "k": (Wk_sb, KT, bk_sb),
                "v": (Wv_sb, VT, None),
                "q": (Wq_sb, QT, bq_sb),
            }[which]
            ps = pp.tile([128, MMW], F32, name="fill_ps")
            for kc in range(KC):
                nc.tensor.matmul(
                    ps,
                    lhsT=W_sb[:, kc, :],
                    rhs=X[:, kc, mt * MMW:(mt + 1) * MMW],
                    start=(kc == 0),
                    stop=(kc == KC - 1),
                )
            dslice = dst[:, mt * MMW:(mt + 1) * MMW]
            if bias_sb is not None:
                nc.vector.tensor_scalar_add(dslice, ps, bias_sb)
            else:
                nc.vector.tensor_copy(dslice, ps)
            if dst is VT:
                for tl in range(MMW // 128):
                    t = mt_loc * (MMW // 128) + tl
                    v = vp.tile([128, 128], BF16, name="vt")
                    nc.sync.dma_start(
                        v,
                        VT[:, b * M + t * 128: b * M + (t + 1) * 128],
                        transpose=True,
                    )
                    for h in range(HLOC):
                        nc.vector.tensor_copy(
                            V1[:, b, t, h, 0:64], v[:, h * 64:(h + 1) * 64]
                        )

        def attn_qt(b, qt, filler=None):
            """One 512-query tile of attention for batch b. `filler` is a
            deque of small thunks emitting PE work to fill the exp-lag gap;
            they are spread evenly across the score/AV stream."""
            C = C_tiles[b]
            q0 = b * M + qt * MMW
            at_tiles = {}
            cts = [ctp.tile([65, MMW], F32, name="ct") for _ in range(HLOC)]

            def avz(kt):
                # A@V with the ones column: ct row 64 accumulates the
                # softmax denominator Z for free.
                for h in range(HLOC):
                    nc.tensor.matmul(
                        cts[h],
                        lhsT=V1[:, b, kt, h, :],
                        rhs=at_tiles[kt][:, h * MMW:(h + 1) * MMW],
                        start=(kt == 0),
                        stop=(kt == NKT - 1),
                    )

            nfil = len(filler) if filler else 0
            shift = min(2, NKT - 1)
            for kt in range(NKT):
                # paired-head score tile: halves written per head (K=64 pair
                # runs concurrently on disjoint PE row groups); exp per half
                # so subtile WAR tracking releases each half early.
                st = stp.tile([128, HLOC * MMW], F32, name="st")
                for h in range(HLOC):
                    hs = slice(h * 64, (h + 1) * 64)
                    nc.tensor.matmul(
                        st[:, h * MMW:(h + 1) * MMW],
                        lhsT=KT[hs, b * M + kt * 128: b * M + (kt + 1) * 128],
                        rhs=QT[hs, q0: q0 + MMW],
                        start=True,
                        stop=True,
                    )
                c0 = kt * 128 - qt * MMW
                if 0 <= c0 < MMW:
                    for h in range(HLOC):
                        o = h * MMW + c0
                        nc.vector.tensor_mul(
                            st[:, o:o + 128], st[:, o:o + 128], mask_sb
                        )
                at = atp.tile([128, HLOC * MMW], BF16, name="at")
                nc.scalar.activation(at, st, mybir.ActivationFunctionType.Exp)
                at_tiles[kt] = at
                if kt >= shift:
                    avz(kt - shift)
                # spread filler PE work evenly across the qt; the exp stream
                # (ACT) is the bottleneck, this rides in PE slack
                if filler and (kt + 1) * nfil // NKT > kt * nfil // NKT:
                    filler.popleft()()
            for kt in range(NKT - shift, NKT):
                avz(kt)
            while filler:
                filler.popleft()()
            # normalize: C = ct * broadcast(1/Z) (Z in ct row 64)
            for h in range(HLOC):
                rz = rzp.tile([1, MMW], F32, name="rz")
                nc.vector.reciprocal(rz, cts[h][64:65, :])
                rzb = rzbp.tile([64, MMW], F32, name="rzb")
                nc.gpsimd.partition_broadcast(rzb, rz)
                nc.vector.tensor_mul(
                    C[h * 64:(h + 1) * 64, qt * MMW:(qt + 1) * MMW],
                    cts[h][0:64, :], rzb,
                )

        def outproj_unit(b, mt, j):
            """One 128-row x 512-col out-projection tile of batch b."""
            C = C_tiles[b]
            op = opp.tile([128, MMW], F32, name="fill_ps")
            nc.tensor.matmul(
                op,
                lhsT=C[:, mt * 128:(mt + 1) * 128],
                rhs=Wo_sb[:, j * MMW:(j + 1) * MMW],
                start=True,
                stop=True,
            )
            osb = osp.tile([128, MMW], BF16, name="osb")
            nc.vector.tensor_copy(osb, op)
            nc.sync.dma_start(
                out.ap()[b * M + mt * 128: b * M + (mt + 1) * 128,
                         j * MMW:(j + 1) * MMW],
                osb,
            )

        # ---------------- the pipelined schedule ----------------
        for b in range(B):
            C_tiles[b] = cp.tile([128, M], BF16, name="C")

        def P(b, mt_loc, which):
            return lambda: proj_tile(b, mt_loc, which)

        def O(b, mt, j):
            return lambda: outproj_unit(b, mt, j)

        # serial head: projections for batch 0
        for mt_loc in range(NQT):
            for w in ("k", "v", "q"):
                proj_tile(0, mt_loc, w)

        # attn(b0): proj(b1) m-tiles + outproj(b0, qt-1) spliced into the
        # per-qt PE slack (C(b0,qt-1) is complete once qt-1 normalizes)
        MPQ = NMT // NQT
        for qt in range(NQT):
            fill = deque([P(1, qt, "k"), P(1, qt, "v"), P(1, qt, "q")])
            if qt > 0:
                for mt in range((qt - 1) * MPQ, qt * MPQ):
                    for j in range(NJ):
                        fill.append(O(0, mt, j))
            attn_qt(0, qt, filler=fill)

        # attn(b1): rest of outproj(b0) + outproj(b1, qt-1) spliced
        for qt in range(NQT):
            fill = deque()
            if qt == 0:
                for mt in range((NQT - 1) * MPQ, NMT):
                    for j in range(NJ):
                        fill.append(O(0, mt, j))
            else:
                for mt in range((qt - 1) * MPQ, qt * MPQ):
                    for j in range(NJ):
                        fill.append(O(1, mt, j))
            attn_qt(1, qt, filler=fill)

        # tail: final out-proj chunk of batch 1
        for mt in range((NQT - 1) * MPQ, NMT):
            for j in range(NJ):
                outproj_unit(1, mt, j)

        if dbg is not None:
            nc.sync.dma_start(dbg["qt"].ap(), QT)
            nc.sync.dma_start(dbg["kt"].ap(), KT)
            nc.sync.dma_start(dbg["c"].ap()[:, 0:M], C_tiles[0])
            nc.sync.dma_start(dbg["c"].ap()[:, M:S], C_tiles[1])


def build_bass(M, debug=False, reps=1):
    """Build + compile the per-core Bass program (same program on all cores).

    reps > 1 wraps the whole body in an on-device loop — used only for
    timing (amortizes host dispatch overhead over many executions).
    """
    S = B * M
    nc = bacc.Bacc("TRN2", target_bir_lowering=False, debug=False)
    xT = nc.dram_tensor("xT", [D, S], BF16, kind="ExternalInput")
    wqT = nc.dram_tensor("wqT", [D, DLOC], BF16, kind="ExternalInput")
    wkT = nc.dram_tensor("wkT", [D, DLOC], BF16, kind="ExternalInput")
    wvT = nc.dram_tensor("wvT", [D, DLOC], BF16, kind="ExternalInput")
    woT = nc.dram_tensor("woT", [DLOC, D], BF16, kind="ExternalInput")
    bq = nc.dram_tensor("bq", [DLOC, 1], F32, kind="ExternalInput")
    bk = nc.dram_tensor("bk", [DLOC, 1], F32, kind="ExternalInput")
    mask = nc.dram_tensor("mask", [128, 128], F32, kind="ExternalInput")
    out = nc.dram_tensor("out", [S, D], BF16, kind="ExternalOutput")

    dbg = None
    if debug:
        dbg = {
            "qt": nc.dram_tensor("dbg_qt", [128, S], BF16, kind="ExternalOutput"),
            "kt": nc.dram_tensor("dbg_kt", [128, S], BF16, kind="ExternalOutput"),
            "c": nc.dram_tensor("dbg_c", [128, S], BF16, kind="ExternalOutput"),
        }

    with tile.TileContext(nc) as tc:
        if reps > 1:
            with tc.For_i(0, reps, 1):
                emit_kernel(tc, M, xT, wqT, wkT, wvT, woT, bq, bk, mask, out,
                            dbg=dbg)
        else:
            emit_kernel(tc, M, xT, wqT, wkT, wvT, woT, bq, bk, mask, out,
                        dbg=dbg)
    nc.compile()
    return nc


def make_in_maps(M, x, Wq, bq, Wk, bk, Wv, Wo):
    """Host-side sharding: per-core input dicts."""
    S = B * M
    bf = ml_dtypes.bfloat16
    scale = 1.0 / np.sqrt(DH)
    xT = np.ascontiguousarray(x.reshape(S, D).T).astype(bf)
    mask = (1.0 - np.eye(128, dtype=np.float32))
    in_maps = []
    for c in range(NCORES):
        sl = slice(c * DLOC, (c + 1) * DLOC)
        in_maps.append({
            "xT": xT,
            "wqT": np.ascontiguousarray((Wq[sl] * scale).T).astype(bf),
            "wkT": np.ascontiguousarray(Wk[sl].T).astype(bf),
            "wvT": np.ascontiguousarray(Wv[sl].T).astype(bf),
            "woT": np.ascontiguousarray(Wo[:, sl].T).astype(bf),
            "bq": (bq[sl] * scale).reshape(DLOC, 1).astype(np.float32),
            "bk": bk[sl].reshape(DLOC, 1).astype(np.float32),
            "mask": mask,
        })
    return in_maps


_NC_CACHE = {}


def kernel(x, Wq, bq, Wk, bk, Wv, bv, Wo, bo):
    x = np.asarray(x, dtype=np.float32)
    Wq = np.asarray(Wq, dtype=np.float32)
    bq = np.asarray(bq, dtype=np.float32)
    Wk = np.asarray(Wk, dtype=np.float32)
    bk = np.asarray(bk, dtype=np.float32)
    Wv = np.asarray(Wv, dtype=np.float32)
    bv = np.asarray(bv, dtype=np.float32)
    Wo = np.asarray(Wo, dtype=np.float32)
    bo = np.asarray(bo, dtype=np.float32)

    M = x.shape[1]
    if M not in _NC_CACHE:
        _NC_CACHE[M] = build_bass(M)
    nc = _NC_CACHE[M]

    in_maps = make_in_maps(M, x, Wq, bq, Wk, bk, Wv, Wo)
    res = run_bass_kernel_spmd(nc, in_maps, core_ids=list(range(NCORES)))

    out = np.zeros((B * M, D), np.float64)
    for c in range(NCORES):
        out += res.results[c]["out"].astype(np.float64)
    out = out.astype(np.float32)
    out += bv @ Wo.T + bo          # folded bv/bo contribution
    return out.reshape(B, M, D)


# revision 25
# speedup vs baseline: 1.0271x; 1.0271x over previous
"""Multi-head self-attention (diag-zero mask) TRN2 kernel, 8-core head-parallel.

Sharding: 16 heads / 8 cores = 2 heads per core; every core sees the full
sequence (both batches), computes Q/K/V projections for its 2 heads,
attention, and its partial out-projection (Wo rows for its head block).
Host sums the 8 partial outputs (the out_proj all-reduce) and adds biases.

Math notes:
  - 1/sqrt(Dh) folded into Wq/bq on host.
  - scores are computed transposed (keys on partitions, queries on free dim)
    so exp() needs no on-chip reduction.
  - softmax denominator Z comes from a ones-vector matmul over the exp'd
    attention tiles, accumulated in PSUM like the A@V product.
  - diag-zero mask: multiply the score diagonal block by (1-eye) before exp
    (masked score 0 -> exp(0) = 1, matching the reference softmax).
  - bv and bo contributions are rank-1/constant terms folded in on host:
    out += bv @ Wo.T + bo.

Schedule (v4): ACT (exp, ~135us/core) is the target bottleneck; every other
engine's work is pipelined under it.
  - score matmuls are K=64 pairs on disjoint PE row groups (concurrent);
  - A@V matmuls are 64-dim-out pairs on disjoint PE col groups (concurrent);
  - Z matmuls are 1-row-out pairs on disjoint col groups;
  - projections and out-projections are spliced as filler into the
    attention stream's PE slack, so only ~1 m-tile of projection runs
    un-overlapped at kernel start.
Out-projection partials are emitted in bf16 (summed on host in f64).
"""

from collections import deque
from contextlib import ExitStack

import numpy as np
import ml_dtypes

import concourse.bass as bass
import concourse.tile as tile
from concourse import bacc, mybir
from concourse.bass_utils import run_bass_kernel_spmd

BF16 = mybir.dt.bfloat16
F32 = mybir.dt.float32

B = 2
D = 1024
H = 16
DH = 64
NCORES = 8
HLOC = H // NCORES          # 2 heads per core
DLOC = HLOC * DH            # 128 head-dims per core
KC = D // 128               # 8 contraction chunks for projections
MMW = 512                   # matmul moving width (one PSUM bank of f32)


def emit_kernel(tc, M, xT, wqT, wkT, wvT, woT, bq, bk, mask, out, dbg=None):
    """Emit the per-core program. M = per-batch sequence length."""
    nc = tc.nc
    S = B * M               # flattened sequence rows
    NKT = M // 128          # key tiles per batch
    NQT = M // MMW          # 512-wide q tiles per batch
    NMT = M // 128          # 128-row out-proj tiles per batch
    NJ = D // MMW

    with ExitStack() as ctx:
        consts = ctx.enter_context(tc.tile_pool(name="consts", bufs=1))
        QT = consts.tile([128, S], BF16)    # [2 heads x 64 dims, S]
        KT = consts.tile([128, S], BF16)
        Wo_sb = consts.tile([128, D], BF16)
        bq_sb = consts.tile([128, 1], F32)
        bk_sb = consts.tile([128, 1], F32)
        mask_sb = consts.tile([128, 128], F32)

        nc.sync.dma_start(Wo_sb, woT.ap())
        nc.sync.dma_start(bq_sb, bq.ap())
        nc.sync.dma_start(bk_sb, bk.ap())
        nc.sync.dma_start(mask_sb, mask.ap())

        # V natural layout + ones column: staging tiles written by DMA
        # transpose ([128,128] aligned), then DVE-copied into the 65-col
        # V1 layout (ones col appended for the free softmax denominator).
        V1 = consts.tile([128, B, NKT, HLOC, 65], BF16)
        nc.vector.memset(V1[:, :, :, :, 64:65], 1.0)
        vp = ctx.enter_context(tc.tile_pool(name="v_pool", bufs=4))

        # attention pools (live for the whole kernel)
        stp = ctx.enter_context(tc.tile_pool(name="st_psum", bufs=2, space="PSUM"))
        atp = ctx.enter_context(tc.tile_pool(name="at_pool", bufs=NKT + 2))
        ctp = ctx.enter_context(tc.tile_pool(name="ct_psum", bufs=2, space="PSUM"))
        rzp = ctx.enter_context(tc.tile_pool(name="rz_pool", bufs=4))
        rzbp = ctx.enter_context(tc.tile_pool(name="rzb_pool", bufs=4))
        cp = ctx.enter_context(tc.tile_pool(name="c_pool", bufs=B))
        C_tiles = {}

        # projection pools
        proj_sbuf = ctx.enter_context(tc.tile_pool(name="proj_sbuf", bufs=1))
        Wq_sb = proj_sbuf.tile([128, KC, DLOC], BF16)
        Wk_sb = proj_sbuf.tile([128, KC, DLOC], BF16)
        Wv_sb = proj_sbuf.tile([128, KC, DLOC], BF16)
        nc.sync.dma_start(Wk_sb, wkT.ap().rearrange("(c p) d -> p c d", p=128))
        nc.sync.dma_start(Wv_sb, wvT.ap().rearrange("(c p) d -> p c d", p=128))
        nc.sync.dma_start(Wq_sb, wqT.ap().rearrange("(c p) d -> p c d", p=128))
        X = proj_sbuf.tile([128, KC, S], BF16)
        xT_r = xT.ap().rearrange("(c p) m -> p c m", p=128)
        for mt in range(S // MMW):
            nc.sync.dma_start(
                X[:, :, mt * MMW:(mt + 1) * MMW],
                xT_r[:, :, mt * MMW:(mt + 1) * MMW],
            )
        VT = proj_sbuf.tile([128, S], BF16)

        pp = ctx.enter_context(tc.tile_pool(name="fill_psum", bufs=2, space="PSUM"))
        opp = pp
        osp = ctx.enter_context(tc.tile_pool(name="out_sbuf", bufs=4))

        def proj_tile(b, mt_loc, which):
            """One projection (K, V or Q) for one 512-wide m-tile of batch b.
            For V, also the DMA-transposes into the V tiles."""
            mt = b * NQT + mt_loc
            W_sb, dst, bias_sb = {
                "k": (Wk_sb, KT, bk_sb),
                "v": (Wv_sb, VT, None),
                "q": (Wq_sb, QT, bq_sb),
            }[which]
            ps = pp.tile([128, MMW], F32, name="fill_ps")
            for kc in range(KC):
                nc.tensor.matmul(
                    ps,
                    lhsT=W_sb[:, kc, :],
                    rhs=X[:, kc, mt * MMW:(mt + 1) * MMW],
                    start=(kc == 0),
                    stop=(kc == KC - 1),
                )
            dslice = dst[:, mt * MMW:(mt + 1) * MMW]
            if bias_sb is not None:
                nc.vector.tensor_scalar_add(dslice, ps, bias_sb)
            else:
                nc.vector.tensor_copy(dslice, ps)
            if dst is VT:
                for tl in range(MMW // 128):
                    t = mt_loc * (MMW // 128) + tl
                    v = vp.tile([128, 128], BF16, name="vt")
                    nc.sync.dma_start(
                        v,
                        VT[:, b * M + t * 128: b * M + (t + 1) * 128],
                        transpose=True,
                    )
                    for h in range(HLOC):
                        nc.vector.tensor_copy(
                            V1[:, b, t, h, 0:64], v[:, h * 64:(h + 1) * 64]
                        )

        def attn_qt(b, qt, filler=None):
            """One 512-query tile of attention for batch b. `filler` is a
            deque of small thunks emitting PE work to fill the exp-lag gap;
            they are spread evenly across the score/AV stream."""
            C = C_tiles[b]
            q0 = b * M + qt * MMW
            at_tiles = {}
            cts = [ctp.tile([65, MMW], F32, name="ct") for _ in range(HLOC)]

            def avz(kt):
                # A@V with the ones column: ct row 64 accumulates the
                # softmax denominator Z for free.
                for h in range(HLOC):
                    nc.tensor.matmul(
                        cts[h],
                        lhsT=V1[:, b, kt, h, :],
                        rhs=at_tiles[kt][:, h * MMW:(h + 1) * MMW],
                        start=(kt == 0),
                        stop=(kt == NKT - 1),
                    )

            nfil = len(filler) if filler else 0
            shift = min(2, NKT - 1)
            for kt in range(NKT):
                # paired-head score tile: halves written per head (K=64 pair
                # runs concurrently on disjoint PE row groups); exp per half
                # so subtile WAR tracking releases each half early.
                st = stp.tile([128, HLOC * MMW], F32, name="st")
                for h in range(HLOC):
                    hs = slice(h * 64, (h + 1) * 64)
                    nc.tensor.matmul(
                        st[:, h * MMW:(h + 1) * MMW],
                        lhsT=KT[hs, b * M + kt * 128: b * M + (kt + 1) * 128],
                        rhs=QT[hs, q0: q0 + MMW],
                        start=True,
                        stop=True,
                    )
                c0 = kt * 128 - qt * MMW
                if 0 <= c0 < MMW:
                    for h in range(HLOC):
                        o = h * MMW + c0
                        nc.vector.tensor_mul(
                            st[:, o:o + 128], st[:, o:o + 128], mask_sb
                        )
                at = atp.tile([128, HLOC * MMW], BF16, name="at")
                nc.scalar.activation(at, st, mybir.ActivationFunctionType.Exp)
                at_tiles[kt] = at
                if kt >= shift:
                    avz(kt - shift)
                # spread filler PE work evenly across the qt; the exp stream
                # (ACT) is the bottleneck, this rides in PE slack
                if filler and (kt + 1) * nfil // NKT > kt * nfil // NKT:
                    filler.popleft()()
            for kt in range(NKT - shift, NKT):
                avz(kt)
            while filler:
                filler.popleft()()
            # normalize: C = ct * broadcast(1/Z) (Z in ct row 64)
            for h in range(HLOC):
                rz = rzp.tile([1, MMW], F32, name="rz")
                nc.vector.reciprocal(rz, cts[h][64:65, :])
                rzb = rzbp.tile([64, MMW], F32, name="rzb")
                nc.gpsimd.partition_broadcast(rzb, rz)
                nc.vector.tensor_mul(
                    C[h * 64:(h + 1) * 64, qt * MMW:(qt + 1) * MMW],
                    cts[h][0:64, :], rzb,
                )

        def outproj_unit(b, mt, j):
            """One 128-row x 512-col out-projection tile of batch b."""
            C = C_tiles[b]
            op = opp.tile([128, MMW], F32, name="fill_ps")
            nc.tensor.matmul(
                op,
                lhsT=C[:, mt * 128:(mt + 1) * 128],
                rhs=Wo_sb[:, j * MMW:(j + 1) * MMW],
                start=True,
                stop=True,
            )
            osb = osp.tile([128, MMW], BF16, name="osb")
            nc.vector.tensor_copy(osb, op)
            nc.sync.dma_start(
                out.ap()[b * M + mt * 128: b * M + (mt + 1) * 128,
                         j * MMW:(j + 1) * MMW],
                osb,
            )

        # ---------------- the pipelined schedule ----------------
        for b in range(B):
            C_tiles[b] = cp.tile([128, M], BF16, name="C")

        def P(b, mt_loc, which):
            return lambda: proj_tile(b, mt_loc, which)

        def O(b, mt, j):
            return lambda: outproj_unit(b, mt, j)

        # serial head: projections for batch 0
        for mt_loc in range(NQT):
            for w in ("k", "v", "q"):
                proj_tile(0, mt_loc, w)

        # attn(b0): proj(b1) m-tiles spliced into the per-qt PE slack
        for qt in range(NQT):
            attn_qt(0, qt, filler=deque(
                [P(1, qt, "k"), P(1, qt, "v"), P(1, qt, "q")]))

        # attn(b1): outproj(b0) + outproj(b1, qt-1) spliced
        MPQ = NMT // NQT
        for qt in range(NQT):
            fill = deque()
            for mt in range(qt * MPQ, (qt + 1) * MPQ):
                for j in range(NJ):
                    fill.append(O(0, mt, j))
            if qt > 0:
                for mt in range((qt - 1) * MPQ, qt * MPQ):
                    for j in range(NJ):
                        fill.append(O(1, mt, j))
            attn_qt(1, qt, filler=fill)

        # tail: final out-proj chunk of batch 1
        for mt in range((NQT - 1) * MPQ, NMT):
            for j in range(NJ):
                outproj_unit(1, mt, j)

        if dbg is not None:
            nc.sync.dma_start(dbg["qt"].ap(), QT)
            nc.sync.dma_start(dbg["kt"].ap(), KT)
            nc.sync.dma_start(dbg["c"].ap()[:, 0:M], C_tiles[0])
            nc.sync.dma_start(dbg["c"].ap()[:, M:S], C_tiles[1])


def build_bass(M, debug=False, reps=1):
    """Build + compile the per-core Bass program (same program on all cores).

    reps > 1 wraps the whole body in an on-device loop — used only for
    timing (amortizes host dispatch overhead over many executions).
    """
    S = B * M
    nc = bacc.Bacc("TRN2", target_bir_lowering=False, debug=False)
    xT = nc.dram_tensor("xT", [D, S], BF16, kind="ExternalInput")
    wqT = nc.dram_tensor("wqT", [D, DLOC], BF16, kind="ExternalInput")
    wkT = nc.dram_tensor("wkT", [D, DLOC], BF16, kind="ExternalInput")
    wvT = nc.dram_tensor("wvT", [D, DLOC], BF16, kind="ExternalInput")
    woT = nc.dram_tensor("woT", [DLOC, D], BF16, kind="ExternalInput")
    bq = nc.dram_tensor("bq", [DLOC, 1], F32, kind="ExternalInput")
    bk = nc.dram_tensor("bk", [DLOC, 1], F32, kind="ExternalInput")
    mask = nc.dram_tensor("mask", [128, 128], F32, kind="ExternalInput")
    out = nc.dram_tensor("out", [S, D], BF16, kind="ExternalOutput")

    dbg = None
    if debug:
        dbg = {
            "qt": nc.dram_tensor("dbg_qt", [128, S], BF16, kind="ExternalOutput"),
            "kt": nc.dram_tensor("dbg_kt", [128, S], BF16, kind="ExternalOutput"),
            "c": nc.dram_tensor("dbg_c", [128, S], BF16, kind="ExternalOutput"),
        }

    with tile.TileContext(nc) as tc:
        if reps > 1:
            with tc.For_i(0, reps, 1):
                emit_kernel(tc, M, xT, wqT, wkT, wvT, woT, bq, bk, mask, out,
                            dbg=dbg)
        else:
            emit_kernel(tc, M, xT, wqT, wkT, wvT, woT, bq, bk, mask, out,
                        dbg=dbg)
    nc.compile()
    return nc


def make_in_maps(M, x, Wq, bq, Wk, bk, Wv, Wo):
    """Host-side sharding: per-core input dicts."""
    S = B * M
    bf = ml_dtypes.bfloat16
    scale = 1.0 / np.sqrt(DH)
    xT = np.ascontiguousarray(x.reshape(S, D).T).astype(bf)
    mask = (1.0 - np.eye(128, dtype=np.float32))
    in_maps = []
    for c in range(NCORES):
        sl = slice(c * DLOC, (c + 1) * DLOC)
        in_maps.append({
            "xT": xT,
            "wqT": np.ascontiguousarray((Wq[sl] * scale).T).astype(bf),
            "wkT": np.ascontiguousarray(Wk[sl].T).astype(bf),
            "wvT": np.ascontiguousarray(Wv[sl].T).astype(bf),
            "woT": np.ascontiguousarray(Wo[:, sl].T).astype(bf),
            "bq": (bq[sl] * scale).reshape(DLOC, 1).astype(np.float32),
            "bk": bk[sl].reshape(DLOC, 1).astype(np.float32),
            "mask": mask,
        })
    return in_maps


_NC_CACHE = {}


def kernel(x, Wq, bq, Wk, bk, Wv, bv, Wo, bo):
    x = np.asarray(x, dtype=np.float32)
    Wq = np.asarray(Wq, dtype=np.float32)
    bq = np.asarray(bq, dtype=np.float32)
    Wk = np.asarray(Wk, dtype=np.float32)
    bk = np.asarray(bk, dtype=np.float32)
    Wv = np.asarray(Wv, dtype=np.float32)
    bv = np.asarray(bv, dtype=np.float32)
    Wo = np.asarray(Wo, dtype=np.float32)
    bo = np.asarray(bo, dtype=np.float32)

    M = x.shape[1]
    if M not in _NC_CACHE:
        _NC_CACHE[M] = build_bass(M)
    nc = _NC_CACHE[M]

    in_maps = make_in_maps(M, x, Wq, bq, Wk, bk, Wv, Wo)
    res = run_bass_kernel_spmd(nc, in_maps, core_ids=list(range(NCORES)))

    out = np.zeros((B * M, D), np.float64)
    for c in range(NCORES):
        out += res.results[c]["out"].astype(np.float64)
    out = out.astype(np.float32)
    out += bv @ Wo.T + bo          # folded bv/bo contribution
    return out.reshape(B, M, D)


# revision 26
# speedup vs baseline: 1.1187x; 1.0892x over previous
"""Multi-head self-attention (diag-zero mask) TRN2 kernel, 8-core head-parallel.

Sharding: 16 heads / 8 cores = 2 heads per core; every core sees the full
sequence (both batches), computes Q/K/V projections for its 2 heads,
attention, and its partial out-projection (Wo rows for its head block).
Host sums the 8 partial outputs (the out_proj all-reduce) and adds biases.

Math notes:
  - 1/sqrt(Dh) folded into Wq/bq on host.
  - scores are computed transposed (keys on partitions, queries on free dim)
    so exp() needs no on-chip reduction.
  - softmax denominator Z comes from a ones-vector matmul over the exp'd
    attention tiles, accumulated in PSUM like the A@V product.
  - diag-zero mask: multiply the score diagonal block by (1-eye) before exp
    (masked score 0 -> exp(0) = 1, matching the reference softmax).
  - bv and bo contributions are rank-1/constant terms folded in on host:
    out += bv @ Wo.T + bo.

Schedule (v4): ACT (exp, ~135us/core) is the target bottleneck; every other
engine's work is pipelined under it.
  - score matmuls are K=64 pairs on disjoint PE row groups (concurrent);
  - A@V matmuls are 64-dim-out pairs on disjoint PE col groups (concurrent);
  - Z matmuls are 1-row-out pairs on disjoint col groups;
  - projections and out-projections are spliced as filler into the
    attention stream's PE slack, so only ~1 m-tile of projection runs
    un-overlapped at kernel start.
Out-projection partials are emitted in bf16 (summed on host in f64).
"""

from collections import deque
from contextlib import ExitStack

import numpy as np
import ml_dtypes

import concourse.bass as bass
import concourse.tile as tile
from concourse import bacc, mybir
from concourse.bass_utils import run_bass_kernel_spmd

BF16 = mybir.dt.bfloat16
F32 = mybir.dt.float32

B = 2
D = 1024
H = 16
DH = 64
NCORES = 8
HLOC = H // NCORES          # 2 heads per core
DLOC = HLOC * DH            # 128 head-dims per core
KC = D // 128               # 8 contraction chunks for projections
MMW = 512                   # matmul moving width (one PSUM bank of f32)


def emit_kernel(tc, M, xT, wqT, wkT, wvT, woT, bq, bk, mask, out, dbg=None):
    """Emit the per-core program. M = per-batch sequence length."""
    nc = tc.nc
    S = B * M               # flattened sequence rows
    NKT = M // 128          # key tiles per batch
    NQT = M // MMW          # 512-wide q tiles per batch
    NMT = M // 128          # 128-row out-proj tiles per batch
    NJ = D // MMW

    with ExitStack() as ctx:
        consts = ctx.enter_context(tc.tile_pool(name="consts", bufs=1))
        QT = consts.tile([128, S], BF16)    # [2 heads x 64 dims, S]
        KT = consts.tile([128, S], BF16)
        Wo_sb = consts.tile([128, D], BF16)
        bq_sb = consts.tile([128, 1], F32)
        bk_sb = consts.tile([128, 1], F32)
        mask_sb = consts.tile([128, 128], F32)

        nc.sync.dma_start(Wo_sb, woT.ap())
        nc.sync.dma_start(bq_sb, bq.ap())
        nc.sync.dma_start(bk_sb, bk.ap())
        nc.sync.dma_start(mask_sb, mask.ap())

        # V natural layout + ones column: staging tiles written by DMA
        # transpose ([128,128] aligned), then DVE-copied into the 65-col
        # V1 layout (ones col appended for the free softmax denominator).
        V1 = consts.tile([128, B, NKT, HLOC, 65], BF16)
        nc.vector.memset(V1[:, :, :, :, 64:65], 1.0)
        vp = ctx.enter_context(tc.tile_pool(name="v_pool", bufs=4))

        # attention pools (live for the whole kernel)
        stp = ctx.enter_context(tc.tile_pool(name="st_psum", bufs=2, space="PSUM"))
        atp = ctx.enter_context(tc.tile_pool(name="at_pool", bufs=NKT + 2))
        ctp = ctx.enter_context(tc.tile_pool(name="ct_psum", bufs=2, space="PSUM"))
        rzp = ctx.enter_context(tc.tile_pool(name="rz_pool", bufs=4))
        rzbp = ctx.enter_context(tc.tile_pool(name="rzb_pool", bufs=4))
        cp = ctx.enter_context(tc.tile_pool(name="c_pool", bufs=B))
        C_tiles = {}

        # projection pools
        proj_sbuf = ctx.enter_context(tc.tile_pool(name="proj_sbuf", bufs=1))
        Wq_sb = proj_sbuf.tile([128, KC, DLOC], BF16)
        Wk_sb = proj_sbuf.tile([128, KC, DLOC], BF16)
        Wv_sb = proj_sbuf.tile([128, KC, DLOC], BF16)
        nc.sync.dma_start(Wk_sb, wkT.ap().rearrange("(c p) d -> p c d", p=128))
        nc.sync.dma_start(Wv_sb, wvT.ap().rearrange("(c p) d -> p c d", p=128))
        nc.sync.dma_start(Wq_sb, wqT.ap().rearrange("(c p) d -> p c d", p=128))
        X = proj_sbuf.tile([128, KC, S], BF16)
        xT_r = xT.ap().rearrange("(c p) m -> p c m", p=128)
        for mt in range(S // MMW):
            nc.sync.dma_start(
                X[:, :, mt * MMW:(mt + 1) * MMW],
                xT_r[:, :, mt * MMW:(mt + 1) * MMW],
            )
        VT = proj_sbuf.tile([128, S], BF16)

        pp = ctx.enter_context(tc.tile_pool(name="fill_psum", bufs=2, space="PSUM"))
        opp = pp
        osp = ctx.enter_context(tc.tile_pool(name="out_sbuf", bufs=4))

        def proj_tile(b, mt_loc, which):
            """One projection (K, V or Q) for one 512-wide m-tile of batch b.
            For V, also the DMA-transposes into the V tiles."""
            mt = b * NQT + mt_loc
            W_sb, dst, bias_sb = {
                "k": (Wk_sb, KT, bk_sb),
                "v": (Wv_sb, VT, None),
                "q": (Wq_sb, QT, bq_sb),
            }[which]
            ps = pp.tile([128, MMW], F32, name="fill_ps")
            for kc in range(KC):
                nc.tensor.matmul(
                    ps,
                    lhsT=W_sb[:, kc, :],
                    rhs=X[:, kc, mt * MMW:(mt + 1) * MMW],
                    start=(kc == 0),
                    stop=(kc == KC - 1),
                )
            dslice = dst[:, mt * MMW:(mt + 1) * MMW]
            if bias_sb is not None:
                nc.vector.tensor_scalar_add(dslice, ps, bias_sb)
            else:
                nc.vector.tensor_copy(dslice, ps)
            if dst is VT:
                for tl in range(MMW // 128):
                    t = mt_loc * (MMW // 128) + tl
                    v = vp.tile([128, 128], BF16, name="vt")
                    nc.sync.dma_start(
                        v,
                        VT[:, b * M + t * 128: b * M + (t + 1) * 128],
                        transpose=True,
                    )
                    for h in range(HLOC):
                        nc.vector.tensor_copy(
                            V1[:, b, t, h, 0:64], v[:, h * 64:(h + 1) * 64]
                        )

        def attn_qt(b, qt, filler=None):
            """One 512-query tile of attention for batch b. `filler` is a
            deque of small thunks emitting PE work to fill the exp-lag gap;
            they are spread evenly across the score/AV stream."""
            C = C_tiles[b]
            q0 = b * M + qt * MMW
            at_tiles = {}
            cts = [ctp.tile([65, MMW], F32, name="ct") for _ in range(HLOC)]

            def avz(kt):
                # A@V with the ones column: ct row 64 accumulates the
                # softmax denominator Z for free.
                for h in range(HLOC):
                    nc.tensor.matmul(
                        cts[h],
                        lhsT=V1[:, b, kt, h, :],
                        rhs=at_tiles[kt][:, h * MMW:(h + 1) * MMW],
                        start=(kt == 0),
                        stop=(kt == NKT - 1),
                    )

            nfil = len(filler) if filler else 0
            shift = min(3, NKT - 1)
            for kt in range(NKT):
                # paired-head score tile: halves written per head (K=64 pair
                # runs concurrently on disjoint PE row groups); exp per half
                # so subtile WAR tracking releases each half early.
                st = stp.tile([128, HLOC * MMW], F32, name="st")
                for h in range(HLOC):
                    hs = slice(h * 64, (h + 1) * 64)
                    nc.tensor.matmul(
                        st[:, h * MMW:(h + 1) * MMW],
                        lhsT=KT[hs, b * M + kt * 128: b * M + (kt + 1) * 128],
                        rhs=QT[hs, q0: q0 + MMW],
                        start=True,
                        stop=True,
                    )
                c0 = kt * 128 - qt * MMW
                if 0 <= c0 < MMW:
                    for h in range(HLOC):
                        o = h * MMW + c0
                        nc.vector.tensor_mul(
                            st[:, o:o + 128], st[:, o:o + 128], mask_sb
                        )
                at = atp.tile([128, HLOC * MMW], BF16, name="at")
                nc.scalar.activation(at, st, mybir.ActivationFunctionType.Exp)
                at_tiles[kt] = at
                if kt >= shift:
                    avz(kt - shift)
                # spread filler PE work evenly across the qt; the exp stream
                # (ACT) is the bottleneck, this rides in PE slack
                if filler and (kt + 1) * nfil // NKT > kt * nfil // NKT:
                    filler.popleft()()
            for kt in range(NKT - shift, NKT):
                avz(kt)
            while filler:
                filler.popleft()()
            # normalize: C = ct * broadcast(1/Z) (Z in ct row 64)
            for h in range(HLOC):
                rz = rzp.tile([1, MMW], F32, name="rz")
                nc.vector.reciprocal(rz, cts[h][64:65, :])
                rzb = rzbp.tile([64, MMW], F32, name="rzb")
                nc.gpsimd.partition_broadcast(rzb, rz)
                nc.vector.tensor_mul(
                    C[h * 64:(h + 1) * 64, qt * MMW:(qt + 1) * MMW],
                    cts[h][0:64, :], rzb,
                )

        def outproj_unit(b, mt, j):
            """One 128-row x 512-col out-projection tile of batch b."""
            C = C_tiles[b]
            op = opp.tile([128, MMW], F32, name="fill_ps")
            nc.tensor.matmul(
                op,
                lhsT=C[:, mt * 128:(mt + 1) * 128],
                rhs=Wo_sb[:, j * MMW:(j + 1) * MMW],
                start=True,
                stop=True,
            )
            osb = osp.tile([128, MMW], BF16, name="osb")
            nc.vector.tensor_copy(osb, op)
            nc.sync.dma_start(
                out.ap()[b * M + mt * 128: b * M + (mt + 1) * 128,
                         j * MMW:(j + 1) * MMW],
                osb,
            )

        # ---------------- the pipelined schedule ----------------
        for b in range(B):
            C_tiles[b] = cp.tile([128, M], BF16, name="C")

        def P(b, mt_loc, which):
            return lambda: proj_tile(b, mt_loc, which)

        def O(b, mt, j):
            return lambda: outproj_unit(b, mt, j)

        # serial head: projections for batch 0
        for mt_loc in range(NQT):
            for w in ("k", "v", "q"):
                proj_tile(0, mt_loc, w)

        # attn(b0): proj(b1) m-tiles spliced into the per-qt PE slack
        for qt in range(NQT):
            attn_qt(0, qt, filler=deque(
                [P(1, qt, "k"), P(1, qt, "v"), P(1, qt, "q")]))

        # attn(b1): outproj(b0) + outproj(b1, qt-1) spliced
        MPQ = NMT // NQT
        for qt in range(NQT):
            fill = deque()
            for mt in range(qt * MPQ, (qt + 1) * MPQ):
                for j in range(NJ):
                    fill.append(O(0, mt, j))
            if qt > 0:
                for mt in range((qt - 1) * MPQ, qt * MPQ):
                    for j in range(NJ):
                        fill.append(O(1, mt, j))
            attn_qt(1, qt, filler=fill)

        # tail: final out-proj chunk of batch 1
        for mt in range((NQT - 1) * MPQ, NMT):
            for j in range(NJ):
                outproj_unit(1, mt, j)

        if dbg is not None:
            nc.sync.dma_start(dbg["qt"].ap(), QT)
            nc.sync.dma_start(dbg["kt"].ap(), KT)
            nc.sync.dma_start(dbg["c"].ap()[:, 0:M], C_tiles[0])
            nc.sync.dma_start(dbg["c"].ap()[:, M:S], C_tiles[1])


def build_bass(M, debug=False, reps=1):
    """Build + compile the per-core Bass program (same program on all cores).

    reps > 1 wraps the whole body in an on-device loop — used only for
    timing (amortizes host dispatch overhead over many executions).
    """
    S = B * M
    nc = bacc.Bacc("TRN2", target_bir_lowering=False, debug=False)
    xT = nc.dram_tensor("xT", [D, S], BF16, kind="ExternalInput")
    wqT = nc.dram_tensor("wqT", [D, DLOC], BF16, kind="ExternalInput")
    wkT = nc.dram_tensor("wkT", [D, DLOC], BF16, kind="ExternalInput")
    wvT = nc.dram_tensor("wvT", [D, DLOC], BF16, kind="ExternalInput")
    woT = nc.dram_tensor("woT", [DLOC, D], BF16, kind="ExternalInput")
    bq = nc.dram_tensor("bq", [DLOC, 1], F32, kind="ExternalInput")
    bk = nc.dram_tensor("bk", [DLOC, 1], F32, kind="ExternalInput")
    mask = nc.dram_tensor("mask", [128, 128], F32, kind="ExternalInput")
    out = nc.dram_tensor("out", [S, D], BF16, kind="ExternalOutput")

    dbg = None
    if debug:
        dbg = {
            "qt": nc.dram_tensor("dbg_qt", [128, S], BF16, kind="ExternalOutput"),
            "kt": nc.dram_tensor("dbg_kt", [128, S], BF16, kind="ExternalOutput"),
            "c": nc.dram_tensor("dbg_c", [128, S], BF16, kind="ExternalOutput"),
        }

    with tile.TileContext(nc) as tc:
        if reps > 1:
            with tc.For_i(0, reps, 1):
                emit_kernel(tc, M, xT, wqT, wkT, wvT, woT, bq, bk, mask, out,
                            dbg=dbg)
        else:
            emit_kernel(tc, M, xT, wqT, wkT, wvT, woT, bq, bk, mask, out,
                        dbg=dbg)
    nc.compile()
    return nc


def make_in_maps(M, x, Wq, bq, Wk, bk, Wv, Wo):
    """Host-side sharding: per-core input dicts."""
    S = B * M
    bf = ml_dtypes.bfloat16
    scale = 1.0 / np.sqrt(DH)
    xT = np.ascontiguousarray(x.reshape(S, D).T).astype(bf)
    mask = (1.0 - np.eye(128, dtype=np.float32))
    in_maps = []
    for c in range(NCORES):
        sl = slice(c * DLOC, (c + 1) * DLOC)
        in_maps.append({
            "xT": xT,
            "wqT": np.ascontiguousarray((Wq[sl] * scale).T).astype(bf),
            "wkT": np.ascontiguousarray(Wk[sl].T).astype(bf),
            "wvT": np.ascontiguousarray(Wv[sl].T).astype(bf),
            "woT": np.ascontiguousarray(Wo[:, sl].T).astype(bf),
            "bq": (bq[sl] * scale).reshape(DLOC, 1).astype(np.float32),
            "bk": bk[sl].reshape(DLOC, 1).astype(np.float32),
            "mask": mask,
        })
    return in_maps


_NC_CACHE = {}


def kernel(x, Wq, bq, Wk, bk, Wv, bv, Wo, bo):
    x = np.asarray(x, dtype=np.float32)
    Wq = np.asarray(Wq, dtype=np.float32)
    bq = np.asarray(bq, dtype=np.float32)
    Wk = np.asarray(Wk, dtype=np.float32)
    bk = np.asarray(bk, dtype=np.float32)
    Wv = np.asarray(Wv, dtype=np.float32)
    bv = np.asarray(bv, dtype=np.float32)
    Wo = np.asarray(Wo, dtype=np.float32)
    bo = np.asarray(bo, dtype=np.float32)

    M = x.shape[1]
    if M not in _NC_CACHE:
        _NC_CACHE[M] = build_bass(M)
    nc = _NC_CACHE[M]

    in_maps = make_in_maps(M, x, Wq, bq, Wk, bk, Wv, Wo)
    res = run_bass_kernel_spmd(nc, in_maps, core_ids=list(range(NCORES)))

    out = np.zeros((B * M, D), np.float64)
    for c in range(NCORES):
        out += res.results[c]["out"].astype(np.float64)
    out = out.astype(np.float32)
    out += bv @ Wo.T + bo          # folded bv/bo contribution
    return out.reshape(B, M, D)
